# revision 38
# baseline (speedup 1.0000x reference)
"""Trainium2 Bass kernel for nn_AdvancedFastMQA.

Data-parallel over batch B=8 across 8 NeuronCores (1 element/core), fully
transposed dataflow (no on-device transposes of activations):

  phase 1: K / V^T / Q-head0 projections, interleaved over 32 contraction
           chunks while x streams in (x rings on the sync DMA queue first,
           small weights + consts on the scalar DGE queue in parallel).
  phase 2: per-head software pipeline. Head h's score matmuls are woven into
           head h+1's Q-projection (24 of 32 contraction chunks as fp8
           DoubleRow pair-matmuls, weights carry a 64x host scale divided
           back out in the sigmoid scale; DR pair-matmuls are kept
           consecutive so their 256-col LDWEIGHTS hide under the previous
           pair's stream). Scores are emitted as PAIRS of matmuls into one
           2-bank PSUM tile, sigmoided by a single ACT instruction with a
           strided output AP into paired attn tiles. Window overlap dedup:
           each (q,k) score is computed/sigmoided once. Denominators:
           partial adds split DVE/gpsimd, then one M=1 ones-matmul per
           window; reciprocal_approx_fast; gpsimd partition broadcast; DVE
           normalize + overlap blending.
  phase 3: output projection, 8 PSUM accumulators, weight slabs prefetched.

All 16-bit tensors are fp16 (same PE/DVE speed as bf16, 8x less rounding
noise, which buys error headroom for the fp8 path).
"""

import sys

for _p in ("/opt/trn_rl_repo", "/opt/pypackages"):
    if _p not in sys.path:
        sys.path.append(_p)

import numpy as np
import ml_dtypes

import concourse.bacc as bacc
import concourse.tile as tile
import concourse.bass as bass
import concourse.mybir as mybir
from concourse.bass_utils import run_bass_kernel_spmd

F16 = mybir.dt.float16
F32 = mybir.dt.float32
F8 = mybir.dt.float8e4
DR = mybir.MatmulPerfMode.DoubleRow
AF = mybir.ActivationFunctionType

B, S, HD = 8, 1024, 4096
H, DH = 32, 128
WINDOW = 512
SCALE = 1.0 / float(np.sqrt(DH))
ROPE_BASE = 10000.0
NI = HD // 128          # 32 contraction chunks
N8 = 28                 # leading chunks of the Q-proj contraction done in fp8
NB = NI - N8            # fp16 chunks
NT = S // 128           # 8 token chunks
W_STARTS = [0, 256, 512, 768]
W_ENDS = [512, 768, 1024, 1024]

# attn storage: 4 paired tiles [128, 2, width]; per k-chunk (tile, slot, q0)
AT_MAP = {0: (0, 0, 0), 1: (0, 1, 0),
          2: (1, 0, 0), 3: (1, 1, 0),
          4: (2, 0, 256), 5: (2, 1, 256),
          6: (3, 0, 512), 7: (3, 1, 512)}
AT_W = [512, 768, 768, 512]
# score pair emissions: (kc_a, kc_b, q_start, q_len); kc_a/kc_b share a tile
SC_PAIRS = [(0, 1, 0, 512), (2, 3, 0, 512), (2, 3, 512, 256),
            (4, 5, 256, 512), (4, 5, 768, 256), (6, 7, 512, 512)]
# AV / denom operands per window: (kc, offset into the chunk's covered range)
AV_OPS = {
    0: [(0, 0), (1, 0), (2, 0), (3, 0)],
    1: [(2, 256), (3, 256), (4, 0), (5, 0)],
    2: [(4, 256), (5, 256), (6, 0), (7, 0)],
    3: [(6, 256), (7, 256)],
}

_CACHE = {}


def _rope_cache_np(S_, D_, base=ROPE_BASE):
    inv_freq = 1.0 / (base ** (np.arange(0, D_, 2, dtype=np.float32) / D_))
    t = np.arange(S_, dtype=np.float32)
    f = np.outer(t, inv_freq)
    cos = np.zeros((S_, D_), dtype=np.float32)
    sin = np.zeros((S_, D_), dtype=np.float32)
    cos[:, 0::2] = np.cos(f)
    cos[:, 1::2] = np.cos(f)
    sin[:, 0::2] = np.sin(f)
    sin[:, 1::2] = np.sin(f)
    return cos, sin


def build_nc():
    nc = bacc.Bacc("TRN2", debug=False, target_bir_lowering=False)

    xT_d = nc.dram_tensor("xT", [128, NI * S], F16, kind="ExternalInput").ap()
    wqb_d = nc.dram_tensor("wqb", [H, 128, NB * 128], F16, kind="ExternalInput").ap()
    wq8_d = nc.dram_tensor("wq8", [H, 128, N8 * 128], F8, kind="ExternalInput").ap()
    wk_d = nc.dram_tensor("wk", [128, HD], F16, kind="ExternalInput").ap()
    wv_d = nc.dram_tensor("wv", [128, HD], F16, kind="ExternalInput").ap()
    wo_d = nc.dram_tensor("wo", [8, 4, 128, 8 * 512], F16, kind="ExternalInput").ap()
    cos_d = nc.dram_tensor("cosT", [128, S], F16, kind="ExternalInput").ap()
    sin_d = nc.dram_tensor("sinS", [128, S], F16, kind="ExternalInput").ap()
    alpha_d = nc.dram_tensor("alphaB", [128, 256], F16, kind="ExternalInput").ap()
    rotm_d = nc.dram_tensor("rotm", [128, 128], F16, kind="ExternalInput").ap()
    iden_d = nc.dram_tensor("ident", [128, 128], F16, kind="ExternalInput").ap()
    y_d = nc.dram_tensor("y", [S, HD], F32, kind="ExternalOutput").ap()

    with tile.TileContext(nc) as tc:
        with tc.tile_pool(name="consts", bufs=1) as cp:
            xt = cp.tile([128, NB * S], F16)               # x chunks N8..NI
            xt8 = cp.tile([128, N8, S], F8)                # x chunks 0..N8, fp8
            cos_t = cp.tile([128, S], F16)
            sin_t = cp.tile([128, S], F16)
            alpha_t = cp.tile([128, 256], F16)
            ones_t = cp.tile([128, 1], F16)
            nc.vector.memset(ones_t[:], 1.0)
            iden_t = cp.tile([128, 128], F16)
            rotm_t = cp.tile([128, 128], F16)

            kr_t = cp.tile([128, S], F16)                  # roped K
            v_all = cp.tile([128, NT * 128], F16)          # V as 8 lhsT tiles
            ao = cp.tile([128, H * S], F16)                # attention out

            def rope(dst, src, wpool, ppool, ptag, wtag, pbufs=2, wbufs=2):
                for rh in range(2):
                    sl = slice(rh * 512, (rh + 1) * 512)
                    nc.vector.tensor_mul(dst[:, sl], src[:, sl], cos_t[:, sl])
                    rp = ppool.tile([128, 512], F32, tag=ptag, bufs=pbufs, name="rp")
                    nc.tensor.matmul(
                        rp[:], lhsT=rotm_t[:], rhs=src[:, sl], start=True, stop=True
                    )
                    ms = wpool.tile([128, 512], F16, tag=wtag, bufs=wbufs, name="ms")
                    nc.vector.tensor_mul(ms[:], rp[:], sin_t[:, sl])
                    nc.vector.tensor_add(dst[:, sl], dst[:, sl], ms[:])

            with tc.tile_pool(name="work", bufs=1) as wp:
              # ---- phase 1 ----
              with tc.tile_pool(name="p1", bufs=1) as p1:
                with tc.tile_pool(name="ps1", bufs=1, space="PSUM") as pp1:
                    # K/V weights first on the scalar queue: the very first
                    # matmul needs wk_t, so it must not queue behind the rings
                    wk_t = p1.tile([128, HD], F16)
                    nc.scalar.dma_start(out=wk_t[:], in_=wk_d[:])
                    wv_t = p1.tile([128, HD], F16)
                    nc.scalar.dma_start(out=wv_t[:], in_=wv_d[:])
                    rings = []
                    for g in range(N8 // 2):
                        rg = p1.tile([128, 2 * S], F16, tag="xs", bufs=3, name=f"ring{g}")
                        eng = nc.sync if g % 2 == 0 else nc.scalar
                        eng.dma_start(out=rg[:], in_=xT_d[:, g * 2 * S:(g + 1) * 2 * S])
                        rings.append(rg)
                    # head-0 Q weights ride the sync queue after the rings
                    wq_next = wp.tile([128, NB * 128], F16, tag="wq", bufs=2, name="wqt")
                    nc.sync.dma_start(out=wq_next[:], in_=wqb_d[0])
                    wq8_next = wp.tile([128, N8, 128], F8, tag="wq8", bufs=2, name="wq8t")
                    nc.sync.dma_start(out=wq8_next[:], in_=wq8_d[0])
                    for g in range(NB // 4):
                        gs = g * 4 * S
                        nc.scalar.dma_start(out=xt[:, gs:gs + 4 * S],
                                            in_=xT_d[:, N8 * S + gs:N8 * S + gs + 4 * S])
                    nc.scalar.dma_start(out=cos_t[:], in_=cos_d[:])
                    nc.scalar.dma_start(out=sin_t[:], in_=sin_d[:])
                    nc.scalar.dma_start(out=alpha_t[:], in_=alpha_d[:])
                    nc.scalar.dma_start(out=iden_t[:], in_=iden_d[:])
                    nc.scalar.dma_start(out=rotm_t[:], in_=rotm_d[:])

                    kraw = p1.tile([128, S], F16, tag="xst", bufs=3, name="kraw")
                    vtraw = p1.tile([128, S], F16, tag="xst", bufs=3, name="vtraw")

                    kps = [pp1.tile([128, 512], F32, tag="p1k", bufs=2, name=f"kp{hh}") for hh in range(2)]
                    vps = [pp1.tile([128, 512], F32, tag="p1v", bufs=2, name=f"vp{hh}") for hh in range(2)]
                    for i in range(NI):
                        st_ = (i == 0)
                        sp_ = (i == NI - 1)
                        if i < N8:
                            g, ic = divmod(i, 2)
                            xsrc = rings[g][:, ic * S:(ic + 1) * S]
                        else:
                            xsrc = xt[:, (i - N8) * S:(i - N8 + 1) * S]
                        for hh in range(2):
                            rhs = xsrc[:, hh * 512:(hh + 1) * 512]
                            nc.tensor.matmul(kps[hh][:], lhsT=wk_t[:, i * 128:(i + 1) * 128],
                                             rhs=rhs, start=st_, stop=sp_)
                            nc.tensor.matmul(vps[hh][:], lhsT=wv_t[:, i * 128:(i + 1) * 128],
                                             rhs=rhs, start=st_, stop=sp_)
                        if i < N8 and i % 2 == 1:
                            g = i // 2
                            nc.scalar.copy(xt8[:, 2 * g:2 * g + 2, :], rings[g][:])
                    for hh in range(2):
                        nc.scalar.copy(kraw[:, hh * 512:(hh + 1) * 512], kps[hh][:])
                        nc.vector.tensor_copy(vtraw[:, hh * 512:(hh + 1) * 512], vps[hh][:])

                    # prologue: K rope, V transposes (PSUM from the phase-1
                    # pool; runs while the matmul stream drains)
                    rope(kr_t, kraw, p1, pp1, "p1k", "xst", wbufs=3)
                    for t in range(NT):
                        tp = pp1.tile([128, 128], F16, tag="p1v", bufs=2, name=f"vtp{t}")
                        nc.tensor.transpose(tp[:], vtraw[:, t * 128:(t + 1) * 128], iden_t[:])
                        nc.scalar.copy(v_all[:, t * 128:(t + 1) * 128], tp[:])

              # ---- phase 2: head loop (h = -1 projects head 0, no scores) ----
              with tc.tile_pool(name="ps", bufs=1, space="PSUM") as pp:
                for h in range(-1, H):
                    if h >= 0:
                        if h == H - 1:
                            at = [at31[0],
                                  wp.tile([128, 2, AT_W[1]], F16, tag="attn1", bufs=1, name="at1"),
                                  wp.tile([128, 2, AT_W[2]], F16, tag="attn2", bufs=1, name="at2"),
                                  at31[3]]
                        else:
                            at = [wp.tile([128, 2, AT_W[t]], F16, tag=f"attn{t}",
                                          bufs=(2 if t in (0, 3) else 1), name=f"at{t}")
                                  for t in range(4)]
                        aoh = ao[:, h * S:(h + 1) * S]

                        def emit_pair(pi, at=at, qrt=qrt):
                            kca, kcb, qs, qlen = SC_PAIRS[pi]
                            sp = pp.tile([128, 2, 512], F32, tag="pscore", bufs=2, name="sp")
                            nc.tensor.matmul(
                                sp[:, 0, 0:qlen],
                                lhsT=kr_t[:, kca * 128:(kca + 1) * 128],
                                rhs=qrt[:, qs:qs + qlen], start=True, stop=True,
                            )
                            nc.tensor.matmul(
                                sp[:, 1, 0:qlen],
                                lhsT=kr_t[:, kcb * 128:(kcb + 1) * 128],
                                rhs=qrt[:, qs:qs + qlen], start=True, stop=True,
                            )
                            t_i, _, q0 = AT_MAP[kca]
                            nc.scalar.activation(
                                at[t_i][:, :, qs - q0:qs - q0 + qlen],
                                sp[:, :, 0:qlen],
                                AF.Sigmoid, scale=SCALE / 64.0,
                            )

                    # --- score pairs woven into next head's Q proj ---
                    if h + 1 < H:
                        wq_t = wq_next
                        wq8_t = wq8_next
                        qraw = wp.tile([128, S], F16, tag="qraw", bufs=2, name="qraw")
                        k = 0
                        for half in range(2):
                            ps = pp.tile([128, 512], F32, tag="pproj", bufs=2, name="ps")
                            for st_i in range(N8 // 2):
                                nc.tensor.matmul(
                                    ps[:],
                                    lhsT=wq8_t[:, 2 * st_i:2 * st_i + 2, :],
                                    rhs=xt8[:, 2 * st_i:2 * st_i + 2,
                                            half * 512:(half + 1) * 512],
                                    start=(st_i == 0), stop=False,
                                    perf_mode=DR,
                                )
                                if h >= 0 and st_i == 7 and k < 6:
                                    emit_pair(k)
                                    emit_pair(k + 1)
                                    k += 2
                            for i in range(NB):
                                nc.tensor.matmul(
                                    ps[:],
                                    lhsT=wq_t[:, i * 128:(i + 1) * 128],
                                    rhs=xt[:, i * S + half * 512: i * S + (half + 1) * 512],
                                    start=False,
                                    stop=(i == NB - 1),
                                )
                                if h >= 0 and half == 0 and i == 1 and k < 6:
                                    emit_pair(k)
                                    emit_pair(k + 1)
                                    k += 2
                            nc.scalar.copy(qraw[:, half * 512:(half + 1) * 512], ps[:])
                        if h >= 0:
                            while k < 6:
                                emit_pair(k)
                                k += 1
                        if h + 2 < H:
                            wq_next = wp.tile([128, NB * 128], F16, tag="wq", bufs=2, name="wqt")
                            nc.sync.dma_start(out=wq_next[:], in_=wqb_d[h + 2])
                            wq8_next = wp.tile([128, N8, 128], F8, tag="wq8", bufs=2, name="wq8t")
                            nc.sync.dma_start(out=wq8_next[:], in_=wq8_d[h + 2])
                        elif h == H - 2:
                            # prefetch the first two O-proj weight slabs
                            wt_pre = []
                            for q4 in range(2):
                                wt = wp.tile([128, 8 * 512], F16, tag="wo", bufs=2, name=f"wo0_{q4}")
                                nc.sync.dma_start(out=wt[:], in_=wo_d[0, q4])
                                wt_pre.append(wt)
                    else:
                        for pi in range(1, 5):
                            emit_pair(pi)

                    if h < 0:
                        # prologue iteration: just rope head 0's Q
                        qrt = wp.tile([128, S], F16, tag="qrt", bufs=2, name="qrt")
                        rope(qrt, qraw, wp, pp, "pscore", "rope_ms")
                        continue

                    # --- denominator partial adds (DVE) ---
                    def atsl(kc, off, LL):
                        t_i, sl, _ = AT_MAP[kc]
                        return at[t_i][:, sl, off:off + LL]

                    partials = []
                    for w in range(4):
                        L = W_ENDS[w] - W_STARTS[w]
                        ops = AV_OPS[w]
                        partial = wp.tile([128, 512], F16, tag="dnpart", bufs=4, name="dnp")
                        (kc0, off0), (kc1, off1) = ops[0], ops[1]
                        nc.vector.tensor_add(partial[:, 0:L], atsl(kc0, off0, L), atsl(kc1, off1, L))
                        for (kc, off) in ops[2:]:
                            nc.vector.tensor_add(partial[:, 0:L], partial[:, 0:L], atsl(kc, off, L))
                        partials.append(partial)

                    # --- rope for head h+1 ---
                    if h + 1 < H:
                        qrt = wp.tile([128, S], F16, tag="qrt", bufs=2, name="qrt")
                        rope(qrt, qraw, wp, pp, "pscore", "rope_ms")

                    # --- denominator ones-matmuls (2 windows per 2-bank tile) ---
                    dns = [pp.tile([128, 2, 512], F32, tag="pscore", bufs=2, name=f"dn{i}")
                           for i in range(2)]
                    for w in range(4):
                        L = W_ENDS[w] - W_STARTS[w]
                        nc.tensor.matmul(
                            dns[w // 2][0:1, w % 2, 0:L], lhsT=ones_t[:],
                            rhs=partials[w][:, 0:L], start=True, stop=True,
                        )

                    # --- AV + normalize + blend ---
                    for w in range(4):
                        st, en = W_STARTS[w], W_ENDS[w]
                        L = en - st
                        ops = AV_OPS[w]
                        op = pp.tile([128, 512], F32, tag="pout", bufs=2, name="op")
                        for j, (kc, off) in enumerate(ops):
                            nc.tensor.matmul(
                                op[:, 0:L],
                                lhsT=v_all[:, kc * 128:(kc + 1) * 128],
                                rhs=atsl(kc, off, L),
                                start=(j == 0), stop=(j == len(ops) - 1),
                            )
                        rc = wp.tile([1, 512], F32, tag="recip", bufs=2, name="rc")
                        nc.vector.reciprocal_approx_fast(
                            rc[:, 0:L], dns[w // 2][0:1, w % 2, 0:L])
                        rc16 = wp.tile([1, 512], F16, tag="recip16", bufs=2, name="rc16")
                        nc.scalar.copy(rc16[:, 0:L], rc[:, 0:L])
                        rb = wp.tile([128, 512], F16, tag="recipb", bufs=2, name="rb")
                        nc.gpsimd.partition_broadcast(rb[:, 0:L], rc16[:, 0:L])
                        if w == 0:
                            nc.vector.tensor_mul(aoh[:, st:en], op[:, 0:L], rb[:, 0:L])
                        else:
                            t1 = wp.tile([128, 256], F16, tag="bl", bufs=3, name="t1")
                            nc.vector.tensor_mul(t1[:], op[:, 0:256], rb[:, 0:256])
                            t2 = wp.tile([128, 256], F16, tag="bl", bufs=3, name="t2")
                            nc.vector.tensor_sub(t2[:], t1[:], aoh[:, st:st + 256])
                            t3 = wp.tile([128, 256], F16, tag="bl", bufs=3, name="t3")
                            nc.vector.tensor_mul(t3[:], t2[:], alpha_t[:])
                            nc.vector.tensor_add(
                                aoh[:, st:st + 256], aoh[:, st:st + 256], t3[:]
                            )
                            if en > st + 256:
                                nc.vector.tensor_mul(
                                    aoh[:, st + 256:en], op[:, 256:L], rb[:, 256:L]
                                )

                    if h == H - 2:
                        # pre-emit the last head's P1/P6 score pairs here so
                        # its iteration has a shorter ACT-gated tail
                        at31 = [wp.tile([128, 2, AT_W[0]], F16, tag="attn0",
                                        bufs=2, name="at31_0"),
                                None, None,
                                wp.tile([128, 2, AT_W[3]], F16, tag="attn3",
                                        bufs=2, name="at31_3")]
                        emit_pair(0, at=at31, qrt=qrt)
                        emit_pair(5, at=at31, qrt=qrt)

              # ---- phase 3: output projection ----
              with tc.tile_pool(name="ops", bufs=1, space="PSUM") as opp:
                    for oc in range(8):
                        yts = []
                        for _t in range(8):
                            ypt = opp.tile([128, 512], F32, tag="yps", bufs=8, name=f"yps{oc}_{_t}")
                            yts.append(ypt)
                        for q4 in range(4):
                            if oc == 0 and q4 < 2:
                                wt = wt_pre[q4]
                            else:
                                wt = wp.tile([128, 8 * 512], F16, tag="wo", bufs=2, name=f"wo{oc}_{q4}")
                                nc.sync.dma_start(out=wt[:], in_=wo_d[oc, q4])
                            for tc_ in range(8):
                                for ih in range(8):
                                    i = q4 * 8 + ih
                                    nc.tensor.matmul(
                                        yts[tc_][:],
                                        lhsT=ao[:, i * S + tc_ * 128: i * S + tc_ * 128 + 128],
                                        rhs=wt[:, ih * 512:(ih + 1) * 512],
                                        start=(i == 0),
                                        stop=(i == NI - 1),
                                    )
                                if q4 == 3:
                                    yt = wp.tile([128, 512], F32, tag=("qraw" if tc_ % 2 == 0 else "qrt"), bufs=2, name=f"ysb{oc}_{tc_}")
                                    if tc_ % 2 == 0:
                                        nc.scalar.copy(yt[:], yts[tc_][:])
                                    else:
                                        nc.vector.tensor_copy(yt[:], yts[tc_][:])
                                    nc.sync.dma_start(
                                        out=y_d[tc_ * 128:(tc_ + 1) * 128, oc * 512:(oc + 1) * 512],
                                        in_=yt[:],
                                    )
    nc.compile()
    return nc


def prep_inputs(x, Wq, Wk, Wv, Wo):
    """Host-side: transpose/tile/cast so every device DMA is contiguous."""
    hf = np.float16
    f8 = ml_dtypes.float8_e4m3fn
    xr = np.ascontiguousarray(
        x.reshape(B, S, NI, 128).transpose(0, 3, 2, 1).reshape(B, 128, NI * S)
    )
    xT = xr.astype(hf)
    wq_full = np.ascontiguousarray(
        (64.0 * Wq).reshape(H, 128, NI, 128).transpose(0, 3, 2, 1).reshape(H, 128, HD)
    )
    wq8 = np.clip(wq_full[:, :, :N8 * 128], -240.0, 240.0).astype(f8)
    wqb = wq_full[:, :, N8 * 128:].astype(hf)
    wk = np.ascontiguousarray(
        Wk.reshape(128, NI, 128).transpose(2, 1, 0).reshape(128, HD)
    ).astype(hf)
    wv = np.ascontiguousarray(
        Wv.reshape(128, NI, 128).transpose(2, 1, 0).reshape(128, HD)
    ).astype(hf)
    wo = np.ascontiguousarray(
        Wo.reshape(8, 512, 4, 8, 128).transpose(0, 2, 4, 3, 1).reshape(8, 4, 128, 8 * 512)
    ).astype(hf)
    cos, sin = _rope_cache_np(S, DH)
    cosT = np.ascontiguousarray(cos.T).astype(hf)
    sinS = np.ascontiguousarray(sin.T).astype(hf)
    rotm = np.zeros((128, 128), dtype=np.float32)
    rotm[np.arange(64) + 64, np.arange(64)] = -1.0
    rotm[np.arange(64), np.arange(64) + 64] = 1.0
    rotm = rotm.astype(hf)
    alphaB = np.tile(
        np.linspace(0.0, 1.0, 256, dtype=np.float32)[None, :], (128, 1)
    ).astype(hf)
    ident = np.eye(128, dtype=np.float32).astype(hf)
    shared = dict(wqb=wqb, wq8=wq8, wk=wk, wv=wv, wo=wo, cosT=cosT,
                  sinS=sinS, alphaB=alphaB, rotm=rotm, ident=ident)
    in_maps = [dict(xT=xT[b], **shared) for b in range(B)]
    return in_maps


def kernel(x, Wq, Wk, Wv, Wo):
    if "nc" not in _CACHE:
        _CACHE["nc"] = build_nc()
    nc = _CACHE["nc"]
    in_maps = prep_inputs(
        np.asarray(x, dtype=np.float32),
        np.asarray(Wq, dtype=np.float32),
        np.asarray(Wk, dtype=np.float32),
        np.asarray(Wv, dtype=np.float32),
        np.asarray(Wo, dtype=np.float32),
    )
    res = run_bass_kernel_spmd(nc, in_maps, core_ids=list(range(B)))
    out = np.stack([np.asarray(res.results[b]["y"]) for b in range(B)], axis=0)
    return out.astype(np.float32)


if __name__ == "__main__":
    rng = np.random.default_rng(0)
    x = rng.standard_normal((B, S, HD), dtype=np.float32)
    Wq = (rng.standard_normal((HD, HD), dtype=np.float32) * 0.02)
    Wk = (rng.standard_normal((DH, HD), dtype=np.float32) * 0.02)
    Wv = (rng.standard_normal((DH, HD), dtype=np.float32) * 0.02)
    Wo = (rng.standard_normal((HD, HD), dtype=np.float32) * 0.02)
    y = kernel(x=x, Wq=Wq, Wk=Wk, Wv=Wv, Wo=Wo)
    print("out", y.shape, y.dtype, float(np.abs(y).mean()))
